# revision 25
# baseline (speedup 1.0000x reference)
"""Transformer decoder layer (pre-norm, self-attn + cross-attn + FFN) on 8
Trainium2 NeuronCores.

Sharding: core c handles batch b = c//2 and the contiguous half of the 1024
target tokens h = c%2 (512 query rows each). K/V work is duplicated within
each batch pair so there are no collectives; every core runs an identical
program on different data. The host rotates the token order per core so that
each core's own tokens are always columns [0, 512) -> one uniform SPMD
program.

On-device layout is feature-major ([d_model, token]) throughout, so no
on-device transposes are needed: the host pre-transposes x / memory / weights
and transposes the output back. Masks are converted to additive fp32 masks on
the host. Matmuls run in float32r (full PE rate at free dim 512).
"""

import numpy as np
from contextlib import ExitStack

import concourse.bass as bass
import concourse.bacc as bacc
import concourse.tile as tile
from concourse import mybir
from concourse.bass_utils import run_bass_kernel_spmd

D = 1024        # d_model
H = 16          # heads
DK = 64         # head dim
DFF = 4096
B = 4
T = 1024        # tgt/src len
OWN = 512       # query rows per core
P = 128         # partitions
NKC = D // P    # 8 feature chunks
NSC = T // P    # 8 s-chunks
NFC = DFF // P  # 32 ffn chunks
EPS = 1e-6

F32 = mybir.dt.float32
F32R = mybir.dt.float32r
AF = mybir.ActivationFunctionType
ALU = mybir.AluOpType


def _r(ap):
    return ap.bitcast(F32R)


# ---------------------------------------------------------------------------
# program builder (identical for every core; only DRAM contents differ)
# ---------------------------------------------------------------------------

def build_program():
    nc = bacc.Bacc(None)
    dr = {}

    def din(name, shape, dt=F32):
        dr[name] = nc.dram_tensor(name, list(shape), dt, kind="ExternalInput")
        return dr[name]

    din("ones_r", [P, P], F32R)
    din("xT", [D, T], F32R)                # batch-b x, transposed, own tokens first
    din("memT", [D, T], F32R)              # memory[b] transposed
    din("maskT", [T, OWN])           # additive tgt mask, [s_rot, q_local]
    din("smask", [P, NSC])           # additive src mask, column-chunked
    for pre in ("sa", "ca"):
        # W^T tiled [kc, c, P, P] (q weights/bias pre-scaled by 1/8 on host)
        din(f"{pre}_wq", [NKC, NKC, P, P], F32R)
        din(f"{pre}_wk", [NKC, NKC, P, P], F32R)
        din(f"{pre}_wv", [NKC, P, D], F32R)  # V weights: W^T row-chunks
        din(f"{pre}_wo", [NKC, NKC, P, P], F32R)
        din(f"{pre}_bq", [P, NKC])       # column-chunked per-partition bias
        din(f"{pre}_bk", [P, NKC])
        din(f"{pre}_bv", [D], F32R)      # placed at partition 64 (outer trick)
        din(f"{pre}_bo", [P, NKC])
    din("w1", [NKC, NFC, P, P], F32R)
    din("w2", [NFC, P, D], F32R)
    din("b1", [P, NFC])
    din("b2", [P, NKC])
    for ln in ("ln1", "ln2", "ln3"):
        din(f"{ln}_w", [P, NKC])
        din(f"{ln}_b", [P, NKC])

    outT = nc.dram_tensor("outT", [D, OWN], F32, kind="ExternalOutput")

    with ExitStack() as ctx:
        tc = ctx.enter_context(tile.TileContext(nc))
        ctx.enter_context(nc.allow_low_precision(
            reason="float32r staging for full-rate PE matmuls"))
        persist = ctx.enter_context(tc.tile_pool(name="persist", bufs=1))

        ones = persist.tile([P, P], F32R, tag="ones", name="ones")
        nc.sync.dma_start(ones[:], dr["ones_r"][:])
        ones_f = persist.tile([P, 1], F32, tag="ones_f", name="ones_f")
        nc.vector.memset(ones_f[:], 1.0)
        eps_11 = persist.tile([1, 1], F32, tag="eps11", name="eps11")
        nc.vector.memset(eps_11[:], EPS)

        def load_pn(name, n):
            t = persist.tile([P, n], F32, tag=name, name=name)
            nc.sync.dma_start(t[:], dr[name][:])
            return t

        bias = {}
        for pre in ("sa", "ca"):
            for nm in ("bq", "bk", "bo"):
                bias[f"{pre}_{nm}"] = load_pn(f"{pre}_{nm}", NKC)
            bv = persist.tile([P, D], F32R, tag=f"{pre}_bv", name=f"{pre}_bv")
            nc.sync.dma_start(bv[64:65, :], dr[f"{pre}_bv"][None, :])
            bias[f"{pre}_bv"] = bv
        bias["b1"] = load_pn("b1", NFC)
        bias["b2"] = load_pn("b2", NKC)
        lnp = {}
        for ln in ("ln1", "ln2", "ln3"):
            lnp[f"{ln}_w"] = load_pn(f"{ln}_w", NKC)
            lnp[f"{ln}_b"] = load_pn(f"{ln}_b", NKC)
        smask = load_pn("smask", NSC)

        # residual (own tokens), lives to the end
        xcur = [persist.tile([P, OWN], F32, tag=f"xc{i}", name=f"xc{i}")
                for i in range(NKC)]

        tmp = ctx.enter_context(tc.tile_pool(name="tmp", bufs=2))
        stats = ctx.enter_context(tc.tile_pool(name="stats", bufs=1))

        # ------------------------------------------------------------------
        def layer_norm(src_blocks, dst_blocks, w_pn, b_pn, ps_st):
            """Feature-major LN. src/dst_blocks: per 512-token block, a list
            of 8 [P, 512] APs (feature chunks)."""
            for sb, db in zip(src_blocks, dst_blocks):
                src_is_r = sb[0].dtype == F32R
                ones_s = ones[:, 0:1] if src_is_r else ones_f[:]
                ps_s = ps_st.tile([1, 512], F32, tag="lns", name="lns")
                ps_q = ps_st.tile([1, 512], F32, tag="lnq", name="lnq")
                for kc in range(NKC):
                    nc.tensor.matmul(ps_s[:], ones_s, sb[kc],
                                     start=(kc == 0), stop=(kc == NKC - 1))
                    sq = tmp.tile([P, 512], F32R, tag="lnsq", name="lnsq")
                    nc.scalar.activation(sq[:], sb[kc], AF.Square)
                    nc.tensor.matmul(ps_q[:], ones[:, 0:1], sq[:],
                                     start=(kc == 0), stop=(kc == NKC - 1))
                s2 = stats.tile([1, 512], F32, tag="ln_s2", name="ln_s2")
                # s2 = (sum/sqrt(D*(D-1)))^2 = sum^2/(D*(D-1))
                nc.scalar.activation(s2[:], ps_s[:], AF.Square,
                                     scale=float(1.0 / np.sqrt(D * (D - 1.0))))
                var = stats.tile([1, 512], F32, tag="ln_var", name="ln_var")
                nc.vector.scalar_tensor_tensor(
                    var[:], ps_q[:], 1.0 / (D - 1.0), s2[:],
                    op0=ALU.mult, op1=ALU.subtract)
                std = stats.tile([1, 512], F32, tag="ln_std", name="ln_std")
                nc.scalar.activation(std[:], var[:], AF.Sqrt)
                nc.scalar.add(std[:], std[:], eps_11[:])
                rstd = stats.tile([1, 512], F32R, tag="ln_rstd", name="ln_rstd")
                nc.vector.reciprocal(rstd[:], std[:])
                mr = stats.tile([1, 512], F32R, tag="ln_mr", name="ln_mr")
                nc.vector.scalar_tensor_tensor(
                    mr[:], ps_s[:], 1.0 / D, rstd[:],
                    op0=ALU.mult, op1=ALU.mult)
                ps_rb = ps_st.tile([P, 512], F32, tag="ln_rb", name="ln_rb")
                nc.tensor.matmul(ps_rb[:], ones[0:1, :], rstd[:],
                                 start=True, stop=True)
                ps_mb = ps_st.tile([P, 512], F32, tag="ln_mb", name="ln_mb")
                nc.tensor.matmul(ps_mb[:], ones[0:1, :], mr[:],
                                 start=True, stop=True)
                for kc in range(NKC):
                    t = tmp.tile([P, 512], F32, tag="lnt", name="lnt")
                    nc.vector.tensor_mul(t[:], sb[kc], ps_rb[:])
                    nc.vector.tensor_sub(t[:], t[:], ps_mb[:])
                    nc.vector.tensor_scalar(
                        db[kc], t[:], w_pn[:, kc:kc + 1], b_pn[:, kc:kc + 1],
                        op0=ALU.mult, op1=ALU.add)

        # ------------------------------------------------------------------
        def proj_fm(wpool, wname, bias_pn, src_blocks, dst, ps_acc):
            """dst[c][:, blk*512:..] = sum_kc W^T[kc,c].T @ src[blk][kc] + b.
            Weights streamed as [P, P] tiles."""
            nblk = len(src_blocks)
            for c in range(NKC):
                pss = [ps_acc.tile([P, 512], F32, tag=f"proj{blk}",
                                   name=f"proj{blk}") for blk in range(nblk)]
                for kc in range(NKC):
                    wt = wpool.tile([P, P], F32R, tag="ws", name="ws")
                    nc.sync.dma_start(wt[:], dr[wname][kc, c])
                    for blk in range(nblk):
                        nc.tensor.matmul(pss[blk][:], wt[:],
                                         src_blocks[blk][kc],
                                         start=(kc == 0), stop=(kc == NKC - 1))
                for blk in range(nblk):
                    nc.scalar.add(dst[c][:, blk * 512:(blk + 1) * 512],
                                  pss[blk][:], bias_pn[:, c:c + 1])

        def proj_tm_vaug(wpool, wname, src_blocks, vaug, ps_acc):
            """Token-major V projection into [P, H, DK+1] aug tiles."""
            for dc in range(2):
                wv = []
                for kc in range(NKC):
                    wt = wpool.tile([P, 512], F32R, tag="wv", name="wv")
                    nc.sync.dma_start(
                        wt[:], dr[wname][kc][:, dc * 512:(dc + 1) * 512])
                    wv.append(wt)
                for st in range(NSC):
                    sb = src_blocks[st // 4]
                    t0 = (st % 4) * P
                    ps = ps_acc.tile([P, 512], F32, tag="proj0", name="proj0")
                    for kc in range(NKC):
                        nc.tensor.matmul(
                            ps[:], sb[kc][:, t0:t0 + P], wv[kc][:],
                            start=(kc == 0), stop=(kc == NKC - 1))
                    nc.vector.tensor_copy(
                        vaug[st][:, 8 * dc:8 * (dc + 1), 0:DK],
                        ps[:].rearrange("p (h d) -> p h d", h=8))

        # ------------------------------------------------------------------
        def attention(KT, QT, vaug, OT, mask_tiles, smask_pn, bv_tile,
                      att_pools):
            ps_sc, ps_av, ps_rb, epool = att_pools
            for c in range(NKC):
                po = [ps_av.tile([DK + 1, 512], F32, tag="po0", name="po0"),
                      ps_av.tile([DK + 1, 512], F32, tag="po1", name="po1")]
                for i in range(NSC):
                    pssc = [ps_sc.tile([P, 512], F32, tag="sc0", name="sc0"),
                            ps_sc.tile([P, 512], F32, tag="sc1", name="sc1")]
                    for h01 in (0, 1):
                        sl = slice(64 * h01, 64 * h01 + 64)
                        nc.tensor.matmul(
                            pssc[h01][:], KT[c][sl, i * P:(i + 1) * P],
                            QT[c][sl, :], start=True, stop=True)
                    for h01 in (0, 1):
                        e = epool.tile([P, 512], F32R, tag="e", name="e")
                        if mask_tiles is not None:
                            tm = epool.tile([P, 512], F32, tag="emask",
                                            name="emask")
                            nc.vector.tensor_add(tm[:], pssc[h01][:],
                                                 mask_tiles[i][:])
                            nc.scalar.activation(e[:], tm[:], AF.Exp)
                        else:
                            nc.scalar.activation(e[:], pssc[h01][:], AF.Exp,
                                                 bias=smask_pn[:, i:i + 1])
                        nc.tensor.matmul(
                            po[h01][:], vaug[i][:, 2 * c + h01, :],
                            e[:], start=(i == 0), stop=(i == NSC - 1))
                for h01 in (0, 1):
                    h = 2 * c + h01
                    sums = epool.tile([P, 512], F32R, tag="sums", name="sums")
                    nc.vector.tensor_copy(sums[64:65, :],
                                          po[h01][DK:DK + 1, :])
                    # O_un += bv (x) sums  (folds the V bias through softmax)
                    nc.tensor.matmul(
                        po[h01][0:DK, :],
                        bv_tile[64:65, DK * h:DK * h + DK],
                        sums[64:65, :], start=False, stop=True,
                        skip_group_check=True)
                    nc.vector.reciprocal(sums[64:65, :], sums[64:65, :])
                    prb = ps_rb.tile([DK, 512], F32, tag="rb", name="rb")
                    nc.tensor.matmul(prb[:], ones[64:65, 0:DK],
                                     sums[64:65, :], start=True, stop=True)
                    # DVE can read only one PSUM operand: stage po via ACT
                    o_un = epool.tile([DK, 512], F32, tag="o_un", name="o_un")
                    nc.scalar.copy(o_un[:], po[h01][0:DK, :])
                    if h01 == 0:
                        nc.vector.tensor_mul(OT[c][0:DK, :], o_un[:], prb[:])
                    else:
                        ot = epool.tile([DK, 512], F32R, tag="ot", name="ot")
                        nc.vector.tensor_mul(ot[:], o_un[:], prb[:])
                        nc.sync.dma_start(OT[c][DK:P, :], ot[:])

        def out_proj_residual(wpool, wname, bias_pn, OT, ps_acc):
            for c2 in range(NKC):
                ps = ps_acc.tile([P, 512], F32, tag="proj0", name="proj0")
                for c in range(NKC):
                    wt = wpool.tile([P, P], F32R, tag="ws", name="ws")
                    nc.sync.dma_start(wt[:], dr[wname][c, c2])
                    nc.tensor.matmul(ps[:], wt[:], OT[c][:],
                                     start=(c == 0), stop=(c == NKC - 1))
                nc.vector.scalar_tensor_tensor(
                    xcur[c2][:], ps[:], bias_pn[:, c2:c2 + 1], xcur[c2][:],
                    op0=ALU.add, op1=ALU.add)

        def attention_block(prefix, KT, QT, vaug, mask_tiles, use_smask,
                            scope):
            """Runs attention + output projection + residual."""
            otp = scope.enter_context(
                tc.tile_pool(name=f"{prefix}_otp", bufs=1))
            OT = [otp.tile([P, OWN], F32R, tag=f"OT{c}", name=f"OT{c}")[:]
                  for c in range(NKC)]
            with ExitStack() as att:
                ps_sc = att.enter_context(
                    tc.tile_pool(name=f"{prefix}_psc", bufs=2, space="PSUM"))
                ps_av = att.enter_context(
                    tc.tile_pool(name=f"{prefix}_pav", bufs=1, space="PSUM"))
                ps_rb = att.enter_context(
                    tc.tile_pool(name=f"{prefix}_prb", bufs=2, space="PSUM"))
                epool = att.enter_context(
                    tc.tile_pool(name=f"{prefix}_ep", bufs=3))
                attention(KT, QT, vaug, OT, mask_tiles,
                          smask if use_smask else None,
                          bias[f"{prefix}_bv"], (ps_sc, ps_av, ps_rb, epool))
            with ExitStack() as op_scope:
                wsp = op_scope.enter_context(
                    tc.tile_pool(name=f"{prefix}_wso", bufs=8))
                pso = op_scope.enter_context(
                    tc.tile_pool(name=f"{prefix}_pso", bufs=4, space="PSUM"))
                out_proj_residual(wsp, f"{prefix}_wo", bias[f"{prefix}_bo"],
                                  OT, pso)

        # ==================================================================
        # phase 1+2: LN1, self-attention
        # ==================================================================
        with ExitStack() as sa:
            big = sa.enter_context(tc.tile_pool(name="sa_big", bufs=1))
            QT = [big.tile([P, OWN], F32R, tag=f"QT{c}", name=f"QT{c}")[:]
                  for c in range(NKC)]
            KT = [big.tile([P, T], F32R, tag=f"KT{c}", name=f"KT{c}")[:]
                  for c in range(NKC)]
            vaug = [big.tile([P, H, DK + 1], F32R, tag=f"V{i}", name=f"V{i}")[:]
                    for i in range(NSC)]
            for i in range(NSC):
                nc.sync.dma_start(vaug[i][:, :, DK], dr["ones_r"][:, 0:H])

            with ExitStack() as ph:
                h1p = ph.enter_context(tc.tile_pool(name="h1", bufs=1))
                h1_blocks = [
                    [h1p.tile([P, 512], F32R, tag=f"h1_{blk}_{kc}",
                              name=f"h1_{blk}_{kc}")[:] for kc in range(NKC)]
                    for blk in range(2)]
                with ExitStack() as lnscope:
                    xin = lnscope.enter_context(
                        tc.tile_pool(name="xin", bufs=1))
                    ps_ln = lnscope.enter_context(
                        tc.tile_pool(name="ps_ln1", bufs=1, space="PSUM"))
                    x_blocks = []
                    for blk in range(2):
                        xb = []
                        for kc in range(NKC):
                            xt = xin.tile([P, 512], F32R, tag=f"x{blk}_{kc}",
                                          name=f"x{blk}_{kc}")
                            nc.sync.dma_start(
                                xt[:], dr["xT"][kc * P:(kc + 1) * P,
                                                blk * 512:(blk + 1) * 512])
                            xb.append(xt[:])
                        x_blocks.append(xb)
                    layer_norm(x_blocks, h1_blocks, lnp["ln1_w"],
                               lnp["ln1_b"], ps_ln)

                with ExitStack() as wscope:
                    wsp = wscope.enter_context(
                        tc.tile_pool(name="sa_ws", bufs=8))
                    psa = wscope.enter_context(
                        tc.tile_pool(name="ps_sap", bufs=4, space="PSUM"))
                    proj_fm(wsp, "sa_wq", bias["sa_bq"], [h1_blocks[0]],
                            QT, psa)
                    proj_fm(wsp, "sa_wk", bias["sa_bk"], h1_blocks, KT, psa)
                    proj_tm_vaug(wsp, "sa_wv", h1_blocks, vaug, psa)

            # residual base (own half of x)
            for kc in range(NKC):
                nc.sync.dma_start(
                    xcur[kc][:],
                    dr["xT"][kc * P:(kc + 1) * P, 0:OWN].bitcast(F32))

            with ExitStack() as mscope:
                mp = mscope.enter_context(tc.tile_pool(name="maskp", bufs=1))
                mask_tiles = []
                for i in range(NSC):
                    mt = mp.tile([P, OWN], F32, tag=f"mask{i}",
                                 name=f"mask{i}")
                    nc.sync.dma_start(mt[:],
                                      dr["maskT"][i * P:(i + 1) * P, :])
                    mask_tiles.append(mt[:])
                attention_block("sa", KT, QT, vaug, mask_tiles, False, mscope)

        # ==================================================================
        # phase 3: cross attention
        # ==================================================================
        with ExitStack() as ca:
            big = ca.enter_context(tc.tile_pool(name="ca_big", bufs=1))
            QT = [big.tile([P, OWN], F32R, tag=f"cQT{c}", name=f"cQT{c}")[:]
                  for c in range(NKC)]
            KT = [big.tile([P, T], F32R, tag=f"cKT{c}", name=f"cKT{c}")[:]
                  for c in range(NKC)]
            vaug = [big.tile([P, H, DK + 1], F32R, tag=f"cV{i}",
                             name=f"cV{i}")[:] for i in range(NSC)]
            for i in range(NSC):
                nc.sync.dma_start(vaug[i][:, :, DK], dr["ones_r"][:, 0:H])

            with ExitStack() as ph:
                h2p = ph.enter_context(tc.tile_pool(name="h2", bufs=1))
                h2 = [h2p.tile([P, OWN], F32R, tag=f"h2_{kc}",
                               name=f"h2_{kc}")[:] for kc in range(NKC)]
                with tc.tile_pool(name="ps_ln2", bufs=1, space="PSUM") as psl:
                    layer_norm([[x[:] for x in xcur]], [h2], lnp["ln2_w"],
                               lnp["ln2_b"], psl)
                with ExitStack() as wscope:
                    wsp = wscope.enter_context(
                        tc.tile_pool(name="ca_ws", bufs=8))
                    pca = wscope.enter_context(
                        tc.tile_pool(name="ps_cap", bufs=4, space="PSUM"))
                    proj_fm(wsp, "ca_wq", bias["ca_bq"], [h2], QT, pca)

            with ExitStack() as mm_scope:
                memp = mm_scope.enter_context(
                    tc.tile_pool(name="mem", bufs=1))
                wsp = mm_scope.enter_context(
                    tc.tile_pool(name="ca_wkv", bufs=8))
                pca = mm_scope.enter_context(
                    tc.tile_pool(name="ps_ckv", bufs=4, space="PSUM"))
                mem_blocks = []
                for blk in range(2):
                    mb = []
                    for kc in range(NKC):
                        mt = memp.tile([P, 512], F32R, tag=f"m{blk}_{kc}",
                                       name=f"m{blk}_{kc}")
                        nc.sync.dma_start(
                            mt[:], dr["memT"][kc * P:(kc + 1) * P,
                                              blk * 512:(blk + 1) * 512])
                        mb.append(mt[:])
                    mem_blocks.append(mb)
                proj_fm(wsp, "ca_wk", bias["ca_bk"], mem_blocks, KT, pca)
                proj_tm_vaug(wsp, "ca_wv", mem_blocks, vaug, pca)

            with ExitStack() as ascope:
                attention_block("ca", KT, QT, vaug, None, True, ascope)

        # ==================================================================
        # phase 4: FFN
        # ==================================================================
        with ExitStack() as ff:
            ap_pool = ff.enter_context(tc.tile_pool(name="aT", bufs=1))
            aT = [ap_pool.tile([P, OWN], F32R, tag=f"aT{i}", name=f"aT{i}")[:]
                  for i in range(NFC)]
            with ExitStack() as ph:
                h3p = ph.enter_context(tc.tile_pool(name="h3", bufs=1))
                h3 = [h3p.tile([P, OWN], F32R, tag=f"h3_{kc}",
                               name=f"h3_{kc}")[:] for kc in range(NKC)]
                with tc.tile_pool(name="ps_ln3", bufs=1, space="PSUM") as psl:
                    layer_norm([[x[:] for x in xcur]], [h3], lnp["ln3_w"],
                               lnp["ln3_b"], psl)
                with ExitStack() as wscope:
                    wsp = wscope.enter_context(
                        tc.tile_pool(name="ff_ws", bufs=8))
                    ps_f1 = wscope.enter_context(
                        tc.tile_pool(name="ps_ff1", bufs=4, space="PSUM"))
                    for cidx in range(NFC):
                        ps = ps_f1.tile([P, 512], F32, tag="proj0",
                                        name="proj0")
                        for kc in range(NKC):
                            wt = wsp.tile([P, P], F32R, tag="ws", name="ws")
                            nc.sync.dma_start(wt[:], dr["w1"][kc, cidx])
                            nc.tensor.matmul(ps[:], wt[:], h3[kc],
                                             start=(kc == 0),
                                             stop=(kc == NKC - 1))
                        nc.scalar.activation(aT[cidx], ps[:], AF.Relu,
                                             bias=bias["b1"][:, cidx:cidx + 1])

            with ExitStack() as yscope:
                w2p = yscope.enter_context(tc.tile_pool(name="w2p", bufs=3))
                ps_y = yscope.enter_context(
                    tc.tile_pool(name="ps_y", bufs=1, space="PSUM"))
                yps = [ps_y.tile([P, 512], F32, tag=f"y{c2}", name=f"y{c2}")
                       for c2 in range(NKC)]
                for kc2 in range(NFC):
                    wt = w2p.tile([P, D], F32R, tag="w2b", name="w2b")
                    nc.sync.dma_start(wt[:], dr["w2"][kc2])
                    for c2 in range(NKC):
                        nc.tensor.matmul(
                            yps[c2][:], wt[:, c2 * P:(c2 + 1) * P],
                            aT[kc2], start=(kc2 == 0),
                            stop=(kc2 == NFC - 1))
                for c2 in range(NKC):
                    nc.vector.scalar_tensor_tensor(
                        xcur[c2][:], yps[c2][:], bias["b2"][:, c2:c2 + 1],
                        xcur[c2][:], op0=ALU.add, op1=ALU.add)

        for c2 in range(NKC):
            nc.sync.dma_start(outT[c2 * P:(c2 + 1) * P, :], xcur[c2][:])

    nc.finalize()
    return nc


# ---------------------------------------------------------------------------
# host side
# ---------------------------------------------------------------------------

def _tile4(wT):
    """[D_in, D_out] -> [D_in/P, D_out/P, P, P] contiguous tiles."""
    di, do = wT.shape
    return np.ascontiguousarray(
        wT.reshape(di // P, P, do // P, P).transpose(0, 2, 1, 3))


def host_prep(inputs):
    f32 = np.float32
    x = np.asarray(inputs["x"], f32)
    mem = np.asarray(inputs["memory"], f32)
    tgt = np.asarray(inputs["tgt_mask"])
    src = np.asarray(inputs["src_mask"])

    add_tgt = (tgt.astype(f32) - 1.0) * 1e9     # [B, T, T]: 0 or -1e9
    add_src = (src.astype(f32) - 1.0) * 1e9     # [B, T]

    shared = {"ones_r": np.ones((P, P), f32)}
    for pre in ("sa", "ca"):
        wq = np.asarray(inputs[f"{pre}_wq"], f32)
        shared[f"{pre}_wq"] = _tile4(wq.T * 0.125)
        shared[f"{pre}_bq"] = np.ascontiguousarray(
            (np.asarray(inputs[f"{pre}_bq"], f32) * 0.125).reshape(NKC, P).T)
        shared[f"{pre}_wk"] = _tile4(np.asarray(inputs[f"{pre}_wk"], f32).T)
        shared[f"{pre}_wo"] = _tile4(np.asarray(inputs[f"{pre}_wo"], f32).T)
        shared[f"{pre}_wv"] = np.ascontiguousarray(
            np.asarray(inputs[f"{pre}_wv"], f32).T.reshape(NKC, P, D))
        for nm in ("bk", "bo"):
            shared[f"{pre}_{nm}"] = np.ascontiguousarray(
                np.asarray(inputs[f"{pre}_{nm}"], f32).reshape(NKC, P).T)
        shared[f"{pre}_bv"] = np.asarray(inputs[f"{pre}_bv"], f32)
    shared["w1"] = _tile4(np.asarray(inputs["ff_w1"], f32).T)
    shared["w2"] = np.ascontiguousarray(
        np.asarray(inputs["ff_w2"], f32).T.reshape(NFC, P, D))
    shared["b1"] = np.ascontiguousarray(
        np.asarray(inputs["ff_b1"], f32).reshape(NFC, P).T)
    shared["b2"] = np.ascontiguousarray(
        np.asarray(inputs["ff_b2"], f32).reshape(NKC, P).T)
    for ln in ("ln1", "ln2", "ln3"):
        shared[f"{ln}_w"] = np.ascontiguousarray(
            np.asarray(inputs[f"{ln}_w"], f32).reshape(NKC, P).T)
        shared[f"{ln}_b"] = np.ascontiguousarray(
            np.asarray(inputs[f"{ln}_b"], f32).reshape(NKC, P).T)

    maps = []
    for c in range(8):
        b, half = c // 2, c % 2
        q0 = half * OWN
        order = np.concatenate(
            [np.arange(q0, q0 + OWN), np.r_[0:q0, q0 + OWN:T]]).astype(
                np.int64)
        m = dict(shared)
        m["xT"] = np.ascontiguousarray(x[b][order].T)
        m["memT"] = np.ascontiguousarray(mem[b].T)
        m["maskT"] = np.ascontiguousarray(add_tgt[b][q0:q0 + OWN][:, order].T)
        m["smask"] = np.ascontiguousarray(add_src[b].reshape(NSC, P).T)
        maps.append(m)
    return maps


def gather(results):
    out = np.zeros((B, T, D), np.float32)
    for c in range(8):
        b, half = c // 2, c % 2
        out[b, half * OWN:(half + 1) * OWN, :] = results[c]["outT"].T
    return out


_NC_CACHE = {}


def kernel(**inputs):
    if "nc" not in _NC_CACHE:
        _NC_CACHE["nc"] = build_program()
    nc = _NC_CACHE["nc"]
    in_maps = host_prep(inputs)
    res = run_bass_kernel_spmd(nc, in_maps, list(range(8)))
    return gather(res.results)


if __name__ == "__main__":
    import reference as ref_mod
    inputs = {k: np.asarray(v) for k, v in ref_mod.setup_inputs().items()}
    expected = np.asarray(ref_mod.reference(**ref_mod.setup_inputs()))
    actual = kernel(**inputs)
    err = np.abs(actual - expected).max()
    rel = err / np.abs(expected).max()
    print("max abs err:", err, "rel:", rel)
